# revision 4
# baseline (speedup 1.0000x reference)
"""Multi-head causal self-attention on 8 Trainium2 NeuronCores.

Problem: B=4, T=2048, C=1024, H=16 heads (DH=64), causal mask, fp32 I/O.

Sharding: core i handles batch b=i//2 and head-group g=i%2 (8 heads).
Per-core compute (bf16 matmuls, fp32 accumulation):
  - QKV projection for its 8 heads:  qT/kT in [d', t] layout, V in [t, d']
    layout with an appended ones-column (gives softmax row-sums for free
    during the AV matmul).
  - Causal attention: S^T = kT^T @ qT per (128-key, 512-query) block,
    exp on ScalarE straight out of PSUM (batched over 2 banks), triangular
    masks applied multiplicatively on VectorE for diagonal blocks, then
    O^T (+row-sums) accumulated in PSUM via the AV matmul.
  - Normalization by reciprocal row-sums (broadcast via a DRAM bounce).
  - Output projection partial product, bias on g=0 cores only, then a
    pair-wise ReduceScatter sums the two head-groups of each batch and
    leaves each core with half the rows of its batch's output.
Host assembles the full [4, 2048, 1024] output from the 8 shards.
"""
import sys

if "/opt/trn_rl_repo" not in sys.path:
    sys.path.insert(0, "/opt/trn_rl_repo")

import numpy as np
import ml_dtypes

import concourse.mybir as mybir
import concourse.tile as tile
from concourse import bacc
from concourse.bass_utils import run_bass_kernel_spmd

B, T, C, H = 4, 2048, 1024, 16
DH = C // H              # 64
HL = H // 2              # 8 heads per core
CL = HL * DH             # 512 local channels
THALF = T // 2           # 1024 rows of output per core after ReduceScatter

bf16 = mybir.dt.bfloat16
f32 = mybir.dt.float32
BF = ml_dtypes.bfloat16

REPLICA_GROUPS = [[0, 1], [2, 3], [4, 5], [6, 7]]


def build_body(nc, tc, ext, pools, rep_tag=""):
    """Emit one full forward pass. `ext` holds external APs, `pools` the
    shared tile pools (so repeated bodies reuse SBUF/PSUM slots)."""
    sb, big, ps, pt_pool, small, dram = (
        pools["sb"], pools["big"], pools["ps"], pools["pt"], pools["small"],
        pools["dram"],
    )
    Exp = mybir.ActivationFunctionType.Exp
    mul = mybir.AluOpType.mult

    xs = ext["xs"]; wq_s = ext["wq_s"]; wk_s = ext["wk_s"]; wv_s = ext["wv_s"]
    wp_s = ext["wp_s"]; bqs = ext["bqs"]; bks = ext["bks"]; bvs = ext["bvs"]
    pbs = ext["pbs"]; tris = ext["tris"]; out_ext = ext["out"]

    # ---- working tiles for this pass ----
    qT = big.tile([128, 4, T], bf16, tag="qT")     # [d' slab, t]
    kT = big.tile([128, 4, T], bf16, tag="kT")
    v_all = big.tile([128, HL, 16, DH + 1], bf16, tag="v")  # [t-chunk part, h, tt, d+1]
    proj_dram = dram.tile([T, C], f32, tag="proj")
    sums_dram = dram.tile([32, 512], f32, tag="sums")
    recips_dram = dram.tile([32, 512], f32, tag="recips")
    rs_out = dram.tile([THALF, C], f32, tag="rs")

    nc.vector.memset(v_all[:, :, :, DH:DH + 1], 1.0)

    # ---- QKV projections ----
    # qT/kT: out[d' 128, t 512] = sum_cc w[:, cc, d'-slab].T @ xT[:, cc, t]
    for n in range(4):                  # t-chunks of 512
        for m in range(4):              # d' slabs of 128
            for w_s, dst, bias in ((wq_s, qT, bqs), (wk_s, kT, bks)):
                ps_t = ps.tile([128, 512], f32, tag="P")
                for cc in range(8):
                    nc.tensor.matmul(
                        ps_t[:],
                        lhsT=w_s[:, cc, m * 128:(m + 1) * 128],
                        rhs=xs[:, cc, n * 512:(n + 1) * 512],
                        start=(cc == 0), stop=(cc == 7),
                    )
                nc.vector.tensor_scalar_add(
                    dst[:, m, n * 512:(n + 1) * 512], ps_t[:], bias[:, m:m + 1])
    # V: out[t 128, d' 512] = sum_cc xT[:, cc, tt].T @ wv[:, cc, :]
    for tt in range(16):
        ps_t = ps.tile([128, 512], f32, tag="P")
        for cc in range(8):
            nc.tensor.matmul(
                ps_t[:],
                lhsT=xs[:, cc, tt * 128:(tt + 1) * 128],
                rhs=wv_s[:, cc, :],
                start=(cc == 0), stop=(cc == 7),
            )
        nc.vector.tensor_tensor(
            v_all[:, :, tt, 0:DH],
            ps_t[:].rearrange("p (h d) -> p h d", h=HL),
            bvs[:].rearrange("p (h d) -> p h d", h=HL),
            mybir.AluOpType.add,
        )

    # ---- attention ----
    for qt in range(4):
        attnT = sb.tile([128, 4, 512], bf16, tag="attnT")
        kmax = 4 * (qt + 1)
        for h in range(HL):
            j, half = h // 2, (h % 2) * 64
            ps_O = ps.tile([128, 512], f32, tag="O")
            for p in range(kmax // 2):
                kc0 = 2 * p
                ps_S = ps.tile([128, 1024], f32, tag="S")
                for e in range(2):
                    kc = kc0 + e
                    nc.tensor.matmul(
                        ps_S[:, e * 512:(e + 1) * 512],
                        lhsT=kT[half:half + 64, j, kc * 128:(kc + 1) * 128],
                        rhs=qT[half:half + 64, j, qt * 512:(qt + 1) * 512],
                        start=True, stop=True,
                    )
                pt = pt_pool.tile([128, 1024], bf16, tag="pt")
                nc.scalar.activation(pt[:], ps_S[:], Exp, scale=DH ** -0.5)
                if kc0 >= 4 * qt:        # diagonal pair -> causal mask
                    dp = (kc0 - 4 * qt) // 2
                    nc.vector.tensor_tensor(
                        pt[:],
                        pt[:],
                        tris[:, 2 * dp:2 * dp + 2, :].rearrange("p a b -> p (a b)"),
                        mul,
                    )
                for e in range(2):
                    kc = kc0 + e
                    nc.tensor.matmul(
                        ps_O[0:DH + 1, :],
                        lhsT=v_all[:, h, kc, :],
                        rhs=pt[:, e * 512:(e + 1) * 512],
                        start=(kc == 0), stop=(kc == kmax - 1),
                    )
            r = qt * 8 + h
            srow = small.tile([1, 512], f32, tag="srow")
            nc.vector.tensor_copy(srow[:], ps_O[DH:DH + 1, :])
            nc.sync.dma_start(out=sums_dram[r:r + 1, :], in_=srow[:])
            nc.vector.tensor_copy(attnT[half:half + 64, j, :], ps_O[0:64, :])

        # normalization: recip of row-sums, broadcast via DRAM bounce
        ss = small.tile([8, 512], f32, tag="ss")
        nc.sync.dma_start(out=ss[:], in_=sums_dram[qt * 8:qt * 8 + 8, :])
        rr = small.tile([8, 512], f32, tag="rr")
        nc.vector.reciprocal(rr[:], ss[:])
        nc.sync.dma_start(out=recips_dram[qt * 8:qt * 8 + 8, :], in_=rr[:])
        for j in range(4):
            rb = small.tile([128, 512], f32, tag="rb")
            r0 = qt * 8 + 2 * j
            nc.sync.dma_start(
                out=rb[0:64, :],
                in_=recips_dram[r0:r0 + 1, :].to_broadcast([64, 512]))
            nc.sync.dma_start(
                out=rb[64:128, :],
                in_=recips_dram[r0 + 1:r0 + 2, :].to_broadcast([64, 512]))
            nc.vector.tensor_tensor(attnT[:, j, :], attnT[:, j, :], rb[:], mul)

        # output projection for this q-tile
        for tt in range(4):
            t0 = qt * 512 + tt * 128
            for cn in range(2):
                ps_P = ps.tile([128, 512], f32, tag="P")
                for j in range(4):
                    nc.tensor.matmul(
                        ps_P[:],
                        lhsT=attnT[:, j, tt * 128:(tt + 1) * 128],
                        rhs=wp_s[:, j, cn * 512:(cn + 1) * 512],
                        start=(j == 0), stop=(j == 3),
                    )
                ot = small.tile([128, 512], f32, tag="ot")
                nc.vector.tensor_tensor(
                    ot[:], ps_P[:], pbs[:, cn * 512:(cn + 1) * 512],
                    mybir.AluOpType.add)
                nc.sync.dma_start(
                    out=proj_dram[t0:t0 + 128, cn * 512:(cn + 1) * 512],
                    in_=ot[:])

    # ---- pairwise ReduceScatter + final output ----
    nc.gpsimd.collective_compute(
        "ReduceScatter",
        mybir.AluOpType.add,
        replica_groups=REPLICA_GROUPS,
        ins=[proj_dram.opt()],
        outs=[rs_out.opt()],
    )
    nc.sync.dma_start(out=out_ext[:], in_=rs_out[:])


def build_graph(reps=1):
    nc = bacc.Bacc("TRN2", target_bir_lowering=False, debug=False, num_devices=8)
    xT_e = nc.dram_tensor("xT", [C, T], bf16, kind="ExternalInput").ap()
    wq_e = nc.dram_tensor("wq", [C, CL], bf16, kind="ExternalInput").ap()
    wk_e = nc.dram_tensor("wk", [C, CL], bf16, kind="ExternalInput").ap()
    wv_e = nc.dram_tensor("wv", [C, CL], bf16, kind="ExternalInput").ap()
    wp_e = nc.dram_tensor("wp", [CL, C], bf16, kind="ExternalInput").ap()
    bq_e = nc.dram_tensor("bq", [128, 4], f32, kind="ExternalInput").ap()
    bk_e = nc.dram_tensor("bk", [128, 4], f32, kind="ExternalInput").ap()
    bv_e = nc.dram_tensor("bv", [1, CL], f32, kind="ExternalInput").ap()
    pb_e = nc.dram_tensor("pb", [1, C], f32, kind="ExternalInput").ap()
    tri_e = nc.dram_tensor("tri", [128, 4, 512], bf16, kind="ExternalInput").ap()
    out_e = nc.dram_tensor("out", [THALF, C], f32, kind="ExternalOutput").ap()

    with tile.TileContext(nc) as tc:
        with (
            tc.tile_pool(name="const", bufs=1) as const,
            tc.tile_pool(name="big", bufs=1) as big,
            tc.tile_pool(name="sb", bufs=2) as sb,
            tc.tile_pool(name="pt", bufs=3) as pt_pool,
            tc.tile_pool(name="small", bufs=2) as small,
            tc.tile_pool(name="ps", bufs=2, space="PSUM") as ps,
            tc.tile_pool(name="dram", bufs=2, space="DRAM") as dram,
        ):
            # load constants once
            xs = const.tile([128, 8, T], bf16, tag="xs")
            for cc in range(8):
                nc.sync.dma_start(
                    out=xs[:, cc, :],
                    in_=xT_e.rearrange("(c p) t -> p c t", p=128)[:, cc, :])
            wq_s = const.tile([128, 8, CL], bf16, tag="wq")
            wk_s = const.tile([128, 8, CL], bf16, tag="wk")
            wv_s = const.tile([128, 8, CL], bf16, tag="wv")
            for w_s, w_e in ((wq_s, wq_e), (wk_s, wk_e), (wv_s, wv_e)):
                nc.sync.dma_start(
                    out=w_s[:], in_=w_e.rearrange("(c p) n -> p c n", p=128))
            wp_s = const.tile([128, 4, C], bf16, tag="wp")
            nc.sync.dma_start(
                out=wp_s[:], in_=wp_e.rearrange("(c p) n -> p c n", p=128))
            bqs = const.tile([128, 4], f32, tag="bq")
            nc.sync.dma_start(out=bqs[:], in_=bq_e)
            bks = const.tile([128, 4], f32, tag="bk")
            nc.sync.dma_start(out=bks[:], in_=bk_e)
            bvs = const.tile([128, CL], f32, tag="bv")
            nc.sync.dma_start(out=bvs[:], in_=bv_e.to_broadcast([128, CL]))
            pbs = const.tile([128, C], f32, tag="pb")
            nc.sync.dma_start(out=pbs[:], in_=pb_e.to_broadcast([128, C]))
            tris = const.tile([128, 4, 512], bf16, tag="tri")
            nc.sync.dma_start(out=tris[:], in_=tri_e)

            ext = dict(xs=xs, wq_s=wq_s, wk_s=wk_s, wv_s=wv_s, wp_s=wp_s,
                       bqs=bqs, bks=bks, bvs=bvs, pbs=pbs, tris=tris,
                       out=out_e)
            pools = dict(sb=sb, big=big, ps=ps, pt=pt_pool, small=small,
                         dram=dram)
            for r in range(reps):
                build_body(nc, tc, ext, pools, rep_tag=str(r))

    nc.compile()
    return nc


def prep_shards(x, qkv_w, qkv_b, proj_w, proj_b):
    """Host-side sharding + layout prep. Returns in_maps for 8 cores."""
    kr = np.arange(128)[:, None]
    qr = np.arange(512)[None, :]
    tri = np.stack([(qr >= kr + d * 128) for d in range(4)], axis=1).astype(BF)

    x = np.asarray(x, np.float32)
    qkv_w = np.asarray(qkv_w, np.float32)
    qkv_b = np.asarray(qkv_b, np.float32)
    proj_w = np.asarray(proj_w, np.float32)
    proj_b = np.asarray(proj_b, np.float32)

    in_maps = []
    for core in range(8):
        b, g = core // 2, core % 2
        hsl = slice(g * CL, (g + 1) * CL)
        wq = qkv_w[0 * C:1 * C][hsl]
        wk = qkv_w[1 * C:2 * C][hsl]
        wv = qkv_w[2 * C:3 * C][hsl]
        in_maps.append({
            "xT": np.ascontiguousarray(x[b].T).astype(BF),
            "wq": np.ascontiguousarray(wq.T).astype(BF),
            "wk": np.ascontiguousarray(wk.T).astype(BF),
            "wv": np.ascontiguousarray(wv.T).astype(BF),
            "wp": np.ascontiguousarray(proj_w[:, hsl].T).astype(BF),
            "bq": np.ascontiguousarray(
                qkv_b[0 * C:1 * C][hsl].reshape(4, 128).T).astype(np.float32),
            "bk": np.ascontiguousarray(
                qkv_b[1 * C:2 * C][hsl].reshape(4, 128).T).astype(np.float32),
            "bv": qkv_b[2 * C:3 * C][hsl].reshape(1, CL).astype(np.float32),
            "pb": (proj_b if g == 0 else np.zeros_like(proj_b)
                   ).reshape(1, C).astype(np.float32),
            "tri": tri,
        })
    return in_maps


def assemble(results):
    out = np.empty((B, T, C), np.float32)
    for b in range(B):
        out[b, :THALF] = results[2 * b]["out"]
        out[b, THALF:] = results[2 * b + 1]["out"]
    return out


_CACHE = {}


def _numpy_fallback(x, qkv_w, qkv_b, proj_w, proj_b, mask):
    x = np.asarray(x, np.float64)
    qkv = x @ np.asarray(qkv_w, np.float64).T + np.asarray(qkv_b, np.float64)
    qkv = qkv.reshape(B, T, 3, H, DH).transpose(2, 0, 3, 1, 4)
    q, k, v = qkv[0], qkv[1], qkv[2]
    att = np.einsum("bhqd,bhkd->bhqk", q, k) * (DH ** -0.5)
    att = np.where(np.asarray(mask), att, -np.inf)
    att = att - att.max(axis=-1, keepdims=True)
    att = np.exp(att)
    att /= att.sum(axis=-1, keepdims=True)
    o = np.einsum("bhqk,bhkd->bhqd", att, v)
    o = o.transpose(0, 2, 1, 3).reshape(B, T, C)
    return (o @ np.asarray(proj_w, np.float64).T
            + np.asarray(proj_b, np.float64)).astype(np.float32)


def kernel(x, qkv_w, qkv_b, proj_w, proj_b, mask):
    causal = np.tril(np.ones((T, T), dtype=bool))
    if not np.array_equal(np.asarray(mask).reshape(T, T), causal):
        return _numpy_fallback(x, qkv_w, qkv_b, proj_w, proj_b, mask)

    if "nc" not in _CACHE:
        _CACHE["nc"] = build_graph(reps=1)
    nc = _CACHE["nc"]
    in_maps = prep_shards(x, qkv_w, qkv_b, proj_w, proj_b)
    res = run_bass_kernel_spmd(nc, in_maps, core_ids=list(range(8)))
    return assemble(res.results)


# revision 22
# speedup vs baseline: 26.8099x; 26.8099x over previous
"""Multi-head causal self-attention on 8 Trainium2 NeuronCores.

Problem: B=4, T=2048, C=1024, H=16 heads (DH=64), causal mask, fp32 I/O.

Sharding: core i handles batch b=i//2 and head-group g=i%2 (8 heads).
Per-core compute (bf16 matmuls, fp32 accumulation):
  - QKV projection for its 8 heads:  qT/kT in [d', t] layout, V in [t, d']
    layout with an appended ones-column (gives softmax row-sums for free
    during the AV matmul).
  - Causal attention: S^T = kT^T @ qT per (128-key, 512-query) block,
    exp on ScalarE straight out of PSUM (batched over 2 banks), triangular
    masks applied multiplicatively on VectorE for diagonal blocks, then
    O^T (+row-sums) accumulated in PSUM via the AV matmul.
  - Normalization by reciprocal row-sums (broadcast via a DRAM bounce).
  - Output projection partial product, bias on g=0 cores only, then a
    pair-wise ReduceScatter sums the two head-groups of each batch and
    leaves each core with half the rows of its batch's output.
Host assembles the full [4, 2048, 1024] output from the 8 shards.
"""
import sys

if "/opt/trn_rl_repo" not in sys.path:
    sys.path.insert(0, "/opt/trn_rl_repo")

import numpy as np
import ml_dtypes

import concourse.mybir as mybir
import concourse.tile as tile
from concourse import bacc
from concourse.bass_utils import run_bass_kernel_spmd

B, T, C, H = 4, 2048, 1024, 16
DH = C // H              # 64
HL = H // 2              # 8 heads per core
CL = HL * DH             # 512 local channels
THALF = T // 2           # 1024 rows of output per core after ReduceScatter

bf16 = mybir.dt.bfloat16
f32 = mybir.dt.float32
BF = ml_dtypes.bfloat16

REPLICA_GROUPS = [[0, 1], [2, 3], [4, 5], [6, 7]]


def build_body(nc, tc, ext, pools, rep_tag=""):
    """Emit one full forward pass. `ext` holds external APs, `pools` the
    shared tile pools (so repeated bodies reuse SBUF/PSUM slots)."""
    sb, big, ps, pt_pool, small, dram = (
        pools["sb"], pools["big"], pools["ps"], pools["pt"], pools["small"],
        pools["dram"],
    )
    Exp = mybir.ActivationFunctionType.Exp
    mul = mybir.AluOpType.mult

    xs = ext["xs"]; wq_s = ext["wq_s"]; wk_s = ext["wk_s"]; wv_s = ext["wv_s"]
    wp_s = ext["wp_s"]; bqs = ext["bqs"]; bks = ext["bks"]; bvs = ext["bvs"]
    pbs = ext["pbs"]; tris = ext["tris"]; sels = ext["sels"]
    out_ext = ext["out"]

    # ---- working tiles for this pass ----
    qT = big.tile([128, 4, T], bf16, tag="qT")     # [d' slab, t]
    kT = big.tile([128, 4, T], bf16, tag="kT")
    v_all = big.tile([128, HL, 16, DH + 1], bf16, tag="v")  # [t-chunk part, h, tt, d+1]
    proj_dram = dram.tile([T, C], f32, tag="proj")
    sums_dram = dram.tile([32, 512], f32, tag="sums")
    if ext.get("norm_dma"):
        recips_dram_t = dram.tile([32, 512], bf16, tag="recips")
        ext["recips_dram"] = recips_dram_t
    rs_out = dram.tile([THALF, C], f32, tag="rs")

    nc.vector.memset(v_all[:, :, :, DH:DH + 1], 1.0)

    # ---- QKV projections ----
    # qT/kT: out[d' 128, t 512] = sum_cc w[:, cc, d'-slab].T @ xT[:, cc, t]
    for n in range(4):                  # t-chunks of 512
        for m in range(4):              # d' slabs of 128
            for w_s, dst, bias in ((wq_s, qT, bqs), (wk_s, kT, bks)):
                ps_t = ps.tile([128, 512], f32, tag="P")
                for cc in range(8):
                    nc.tensor.matmul(
                        ps_t[:],
                        lhsT=w_s[:, cc, m * 128:(m + 1) * 128],
                        rhs=xs[:, cc, n * 512:(n + 1) * 512],
                        start=(cc == 0), stop=(cc == 7),
                    )
                nc.vector.tensor_tensor(
                    dst[:, m, n * 512:(n + 1) * 512], ps_t[:],
                    bias[:, m:m + 1].to_broadcast([128, 512]),
                    mybir.AluOpType.add)
    # V: out[t 128, d' 512] = sum_cc xT[:, cc, tt].T @ wv[:, cc, :]
    for tt in range(16):
        ps_t = ps.tile([128, 512], f32, tag="P")
        for cc in range(8):
            nc.tensor.matmul(
                ps_t[:],
                lhsT=xs[:, cc, tt * 128:(tt + 1) * 128],
                rhs=wv_s[:, cc, :],
                start=(cc == 0), stop=(cc == 7),
            )
        nc.vector.tensor_tensor(
            v_all[:, :, tt, 0:DH],
            ps_t[:].rearrange("p (h d) -> p h d", h=HL),
            bvs[:].rearrange("p (h d) -> p h d", h=HL),
            mybir.AluOpType.add,
        )

    if "attn" in ext.get("skip", ()):
        # sink qkv outputs so DCE keeps the QKV phase
        snk = ext["snk"]
        nc.sync.dma_start(out=snk[0:128, 0:2048], in_=qT[:, 0, :])
        nc.sync.dma_start(out=snk[128:256, 0:2048], in_=kT[:, 0, :])
        nc.sync.dma_start(out=snk[256:384, 0:8320],
                          in_=v_all[:].rearrange("p a b c -> p (a b c)"))
        return

    # ---- attention ----
    for qt in range(4):
        attnT = sb.tile([128, 4, 512], bf16, tag="attnT")
        kmax = 4 * (qt + 1)
        for h in range(HL):
            j, half = h // 2, (h % 2) * 64
            ps_O = ps.tile([128, 512], f32, tag="O")
            for p in range(kmax // 2):
                kc0 = 2 * p
                # widths: diagonal chunks only need the causally-valid
                # query suffix (d = kc - 4*qt -> width 512 - 128*d)
                ws = []
                for e in range(2):
                    d = (kc0 + e) - 4 * qt
                    ws.append(512 if d < 0 else 512 - 128 * d)
                ps_S = ps.tile([128, 1024], f32, tag="S")
                for e in range(2):
                    kc, w = kc0 + e, ws[e]
                    nc.tensor.matmul(
                        ps_S[:, e * 512:e * 512 + w],
                        lhsT=kT[half:half + 64, j, kc * 128:(kc + 1) * 128],
                        rhs=qT[half:half + 64, j,
                               qt * 512 + (512 - w):(qt + 1) * 512],
                        start=True, stop=True,
                    )
                pt = pt_pool.tile([128, 1024], bf16, tag="pt")
                espan = 1024 if ext.get("wide_exp") else 512 + ws[1]
                nc.scalar.activation(pt[:, 0:espan], ps_S[:, 0:espan], Exp,
                                     scale=DH ** -0.5)
                if kc0 >= 4 * qt:        # diagonal pair -> causal mask
                    nc.vector.tensor_tensor(
                        pt[:, 0:espan], pt[:, 0:espan], tris[:, 0:espan], mul)
                for e in range(2):
                    kc, w = kc0 + e, ws[e]
                    nc.tensor.matmul(
                        ps_O[0:DH + 1, 512 - w:512],
                        lhsT=v_all[:, h, kc, :],
                        rhs=pt[:, e * 512:e * 512 + w],
                        start=(kc == 0), stop=(kc == kmax - 1),
                    )
            r = qt * 8 + h
            srow = small.tile([1, 512], f32, tag="srow")
            nc.vector.tensor_copy(srow[:], ps_O[DH:DH + 1, :])
            nc.sync.dma_start(out=sums_dram[r:r + 1, :], in_=srow[:])
            nc.vector.tensor_copy(attnT[half:half + 64, j, :], ps_O[0:64, :])

        if "proj" in ext.get("skip", ()):
            snk = ext["snk"]
            nc.sync.dma_start(out=snk[qt * 128:(qt + 1) * 128, 0:2048],
                              in_=attnT[:].rearrange("p a b -> p (a b)"))
            continue

        # normalization: recip of row-sums, partition-broadcast via a
        # selector matmul (rb[p, q] = recip[sel_row(p), q])
        ss = small.tile([8, 512], f32, tag="ss")
        nc.sync.dma_start(out=ss[:], in_=sums_dram[qt * 8:qt * 8 + 8, :])
        rr = small.tile([8, 512], bf16, tag="rr")
        with nc.allow_low_precision(reason="bf16 recip feeds bf16 matmul"):
            nc.vector.reciprocal(rr[:], ss[:])
        if ext.get("norm_dma"):
            recips_dram = ext["recips_dram"]
            nc.sync.dma_start(out=recips_dram[qt * 8:qt * 8 + 8, :], in_=rr[:])
            for j in range(4):
                rb = small.tile([128, 512], bf16, tag="rb")
                r0 = qt * 8 + 2 * j
                nc.sync.dma_start(
                    out=rb[0:64, :],
                    in_=recips_dram[r0:r0 + 1, :].to_broadcast([64, 512]))
                nc.sync.dma_start(
                    out=rb[64:128, :],
                    in_=recips_dram[r0 + 1:r0 + 2, :].to_broadcast([64, 512]))
                nc.vector.tensor_tensor(attnT[:, j, :], attnT[:, j, :], rb[:],
                                        mul)
        else:
            for j in range(4):
                rb_ps = ps.tile([128, 512], f32, tag="P")
                nc.tensor.matmul(rb_ps[:], lhsT=sels[:, j, :], rhs=rr[:],
                                 start=True, stop=True)
                nc.vector.tensor_tensor(attnT[:, j, :], attnT[:, j, :],
                                        rb_ps[:], mul)

        # output projection for this q-tile
        for tt in range(4):
            t0 = qt * 512 + tt * 128
            for cn in range(2):
                ps_P = ps.tile([128, 512], f32, tag="P")
                for j in range(4):
                    nc.tensor.matmul(
                        ps_P[:],
                        lhsT=attnT[:, j, tt * 128:(tt + 1) * 128],
                        rhs=wp_s[:, j, cn * 512:(cn + 1) * 512],
                        start=(j == 0), stop=(j == 3),
                    )
                ot = small.tile([128, 512], f32, tag="ot")
                nc.vector.tensor_tensor(
                    ot[:], ps_P[:], pbs[:, cn * 512:(cn + 1) * 512],
                    mybir.AluOpType.add)
                nc.sync.dma_start(
                    out=proj_dram[t0:t0 + 128, cn * 512:(cn + 1) * 512],
                    in_=ot[:])

        # pairwise ReduceScatter + output DMA for this q-tile's rows
        if ext.get("single_core") or ext.get("no_rs"):
            nc.sync.dma_start(
                out=out_ext[qt * 256:(qt + 1) * 256, :],
                in_=proj_dram[qt * 512:qt * 512 + 256, :])
        else:
            nc.gpsimd.collective_compute(
                "ReduceScatter",
                mybir.AluOpType.add,
                replica_groups=REPLICA_GROUPS,
                ins=[proj_dram[qt * 512:(qt + 1) * 512, :].opt()],
                outs=[rs_out[qt * 256:(qt + 1) * 256, :].opt()],
            )
            nc.sync.dma_start(
                out=out_ext[qt * 256:(qt + 1) * 256, :],
                in_=rs_out[qt * 256:(qt + 1) * 256, :])


def build_graph(reps=1, single_core=False, no_rs=False, skip=(),
                norm_dma=False, wide_exp=False, loop_n=0,
                pt_bufs=3, sb_bufs=2, small_bufs=2):
    nc = bacc.Bacc("TRN2", target_bir_lowering=False, debug=False,
                   num_devices=1 if single_core else 8)
    xT_e = nc.dram_tensor("xT", [C, T], bf16, kind="ExternalInput").ap()
    wq_e = nc.dram_tensor("wq", [C, CL], bf16, kind="ExternalInput").ap()
    wk_e = nc.dram_tensor("wk", [C, CL], bf16, kind="ExternalInput").ap()
    wv_e = nc.dram_tensor("wv", [C, CL], bf16, kind="ExternalInput").ap()
    wp_e = nc.dram_tensor("wp", [CL, C], bf16, kind="ExternalInput").ap()
    bq_e = nc.dram_tensor("bq", [128, 4], f32, kind="ExternalInput").ap()
    bk_e = nc.dram_tensor("bk", [128, 4], f32, kind="ExternalInput").ap()
    bv_e = nc.dram_tensor("bv", [1, CL], f32, kind="ExternalInput").ap()
    pb_e = nc.dram_tensor("pb", [1, C], f32, kind="ExternalInput").ap()
    tri_e = nc.dram_tensor("tri", [128, 1024], bf16, kind="ExternalInput").ap()
    sel_e = nc.dram_tensor("sel", [8, 4, 128], bf16, kind="ExternalInput").ap()
    out_e = nc.dram_tensor("out", [THALF, C], f32, kind="ExternalOutput").ap()
    snk_e = (nc.dram_tensor("snk", [512, 8320], bf16, kind="ExternalOutput").ap()
             if skip else None)

    with tile.TileContext(nc) as tc:
        with (
            tc.tile_pool(name="const", bufs=1) as const,
            tc.tile_pool(name="big", bufs=1) as big,
            tc.tile_pool(name="sb", bufs=sb_bufs) as sb,
            tc.tile_pool(name="pt", bufs=pt_bufs) as pt_pool,
            tc.tile_pool(name="small", bufs=small_bufs) as small,
            tc.tile_pool(name="ps", bufs=2, space="PSUM") as ps,
            tc.tile_pool(name="dram", bufs=2, space="DRAM") as dram,
        ):
            # load constants once
            xs = const.tile([128, 8, T], bf16, tag="xs")
            for cc in range(8):
                nc.sync.dma_start(
                    out=xs[:, cc, :],
                    in_=xT_e.rearrange("(c p) t -> p c t", p=128)[:, cc, :])
            wq_s = const.tile([128, 8, CL], bf16, tag="wq")
            wk_s = const.tile([128, 8, CL], bf16, tag="wk")
            wv_s = const.tile([128, 8, CL], bf16, tag="wv")
            for w_s, w_e in ((wq_s, wq_e), (wk_s, wk_e), (wv_s, wv_e)):
                for cc in range(8):
                    nc.sync.dma_start(
                        out=w_s[:, cc, :],
                        in_=w_e.rearrange("(c p) n -> p c n", p=128)[:, cc, :])
            wp_s = const.tile([128, 4, C], bf16, tag="wp")
            for cc in range(4):
                nc.sync.dma_start(
                    out=wp_s[:, cc, :],
                    in_=wp_e.rearrange("(c p) n -> p c n", p=128)[:, cc, :])
            bqs = const.tile([128, 4], f32, tag="bq")
            nc.sync.dma_start(out=bqs[:], in_=bq_e)
            bks = const.tile([128, 4], f32, tag="bk")
            nc.sync.dma_start(out=bks[:], in_=bk_e)
            bvs = const.tile([128, CL], f32, tag="bv")
            nc.sync.dma_start(out=bvs[:], in_=bv_e.to_broadcast([128, CL]))
            pbs = const.tile([128, C], f32, tag="pb")
            nc.sync.dma_start(out=pbs[:], in_=pb_e.to_broadcast([128, C]))
            tris = const.tile([128, 1024], bf16, tag="tri")
            nc.sync.dma_start(out=tris[:], in_=tri_e)
            sels = const.tile([8, 4, 128], bf16, tag="sel")
            nc.sync.dma_start(out=sels[:], in_=sel_e)

            ext = dict(xs=xs, wq_s=wq_s, wk_s=wk_s, wv_s=wv_s, wp_s=wp_s,
                       bqs=bqs, bks=bks, bvs=bvs, pbs=pbs, tris=tris,
                       sels=sels, out=out_e, snk=snk_e,
                       single_core=single_core, no_rs=no_rs, skip=skip,
                       norm_dma=norm_dma, wide_exp=wide_exp)
            pools = dict(sb=sb, big=big, ps=ps, pt=pt_pool, small=small,
                         dram=dram)
            if loop_n:
                hints = (mybir.EngineType.PE, mybir.EngineType.DVE,
                         mybir.EngineType.Activation, mybir.EngineType.SP,
                         mybir.EngineType.Pool)
                with tc.For_i(0, loop_n, 1, hint_engines=hints):
                    build_body(nc, tc, ext, pools)
            else:
                for r in range(reps):
                    build_body(nc, tc, ext, pools, rep_tag=str(r))

    nc.compile()
    return nc


def prep_shards(x, qkv_w, qkv_b, proj_w, proj_b):
    """Host-side sharding + layout prep. Returns in_maps for 8 cores."""
    kr = np.arange(128)[:, None]
    qr = np.arange(512)[None, :]
    tri1 = (qr >= kr)
    tri = np.concatenate([tri1, tri1], axis=1).astype(BF)
    sel = np.zeros((8, 4, 128), np.float32)
    for j in range(4):
        sel[2 * j, j, 0:64] = 1.0
        sel[2 * j + 1, j, 64:128] = 1.0
    sel = sel.astype(BF)

    x = np.asarray(x, np.float32)
    qkv_w = np.asarray(qkv_w, np.float32)
    qkv_b = np.asarray(qkv_b, np.float32)
    proj_w = np.asarray(proj_w, np.float32)
    proj_b = np.asarray(proj_b, np.float32)

    in_maps = []
    for core in range(8):
        b, g = core // 2, core % 2
        hsl = slice(g * CL, (g + 1) * CL)
        wq = qkv_w[0 * C:1 * C][hsl]
        wk = qkv_w[1 * C:2 * C][hsl]
        wv = qkv_w[2 * C:3 * C][hsl]
        in_maps.append({
            "xT": np.ascontiguousarray(x[b].T).astype(BF),
            "wq": np.ascontiguousarray(wq.T).astype(BF),
            "wk": np.ascontiguousarray(wk.T).astype(BF),
            "wv": np.ascontiguousarray(wv.T).astype(BF),
            "wp": np.ascontiguousarray(proj_w[:, hsl].T).astype(BF),
            "bq": np.ascontiguousarray(
                qkv_b[0 * C:1 * C][hsl].reshape(4, 128).T).astype(np.float32),
            "bk": np.ascontiguousarray(
                qkv_b[1 * C:2 * C][hsl].reshape(4, 128).T).astype(np.float32),
            "bv": qkv_b[2 * C:3 * C][hsl].reshape(1, CL).astype(np.float32),
            "pb": (proj_b if g == 0 else np.zeros_like(proj_b)
                   ).reshape(1, C).astype(np.float32),
            "tri": tri,
            "sel": sel,
        })
    return in_maps


def assemble(results):
    # chunked ReduceScatter: per q-tile chunk of 512 rows, rank 0 holds the
    # first 256 reduced rows, rank 1 the last 256
    out = np.empty((B, T, C), np.float32)
    for b in range(B):
        lo = results[2 * b]["out"]
        hi = results[2 * b + 1]["out"]
        for qt in range(4):
            out[b, qt * 512:qt * 512 + 256] = lo[qt * 256:(qt + 1) * 256]
            out[b, qt * 512 + 256:(qt + 1) * 512] = hi[qt * 256:(qt + 1) * 256]
    return out


_CACHE = {}


def _numpy_fallback(x, qkv_w, qkv_b, proj_w, proj_b, mask):
    x = np.asarray(x, np.float64)
    qkv = x @ np.asarray(qkv_w, np.float64).T + np.asarray(qkv_b, np.float64)
    qkv = qkv.reshape(B, T, 3, H, DH).transpose(2, 0, 3, 1, 4)
    q, k, v = qkv[0], qkv[1], qkv[2]
    att = np.einsum("bhqd,bhkd->bhqk", q, k) * (DH ** -0.5)
    att = np.where(np.asarray(mask), att, -np.inf)
    att = att - att.max(axis=-1, keepdims=True)
    att = np.exp(att)
    att /= att.sum(axis=-1, keepdims=True)
    o = np.einsum("bhqk,bhkd->bhqd", att, v)
    o = o.transpose(0, 2, 1, 3).reshape(B, T, C)
    return (o @ np.asarray(proj_w, np.float64).T
            + np.asarray(proj_b, np.float64)).astype(np.float32)


def kernel(x, qkv_w, qkv_b, proj_w, proj_b, mask):
    causal = np.tril(np.ones((T, T), dtype=bool))
    if not np.array_equal(np.asarray(mask).reshape(T, T), causal):
        return _numpy_fallback(x, qkv_w, qkv_b, proj_w, proj_b, mask)

    if "nc" not in _CACHE:
        _CACHE["nc"] = build_graph(reps=1)
    nc = _CACHE["nc"]
    in_maps = prep_shards(x, qkv_w, qkv_b, proj_w, proj_b)
    res = run_bass_kernel_spmd(nc, in_maps, core_ids=list(range(8)))
    return assemble(res.results)


# revision 23
# speedup vs baseline: 26.8748x; 1.0024x over previous
"""Multi-head causal self-attention on 8 Trainium2 NeuronCores.

Problem: B=4, T=2048, C=1024, H=16 heads (DH=64), causal mask, fp32 I/O.

Sharding: core i handles batch b=i//2 and head-group g=i%2 (8 heads).
Per-core compute (bf16 matmuls, fp32 accumulation):
  - QKV projection for its 8 heads:  qT/kT in [d', t] layout, V in [t, d']
    layout with an appended ones-column (gives softmax row-sums for free
    during the AV matmul).
  - Causal attention: S^T = kT^T @ qT per (128-key, 512-query) block,
    exp on ScalarE straight out of PSUM (batched over 2 banks), triangular
    masks applied multiplicatively on VectorE for diagonal blocks, then
    O^T (+row-sums) accumulated in PSUM via the AV matmul.
  - Normalization by reciprocal row-sums (broadcast via a DRAM bounce).
  - Output projection partial product, bias on g=0 cores only, then a
    pair-wise ReduceScatter sums the two head-groups of each batch and
    leaves each core with half the rows of its batch's output.
Host assembles the full [4, 2048, 1024] output from the 8 shards.
"""
import sys

if "/opt/trn_rl_repo" not in sys.path:
    sys.path.insert(0, "/opt/trn_rl_repo")

import numpy as np
import ml_dtypes

import concourse.mybir as mybir
import concourse.tile as tile
from concourse import bacc
from concourse.bass_utils import run_bass_kernel_spmd

B, T, C, H = 4, 2048, 1024, 16
DH = C // H              # 64
HL = H // 2              # 8 heads per core
CL = HL * DH             # 512 local channels
THALF = T // 2           # 1024 rows of output per core after ReduceScatter

bf16 = mybir.dt.bfloat16
f32 = mybir.dt.float32
BF = ml_dtypes.bfloat16

REPLICA_GROUPS = [[0, 1], [2, 3], [4, 5], [6, 7]]


def build_body(nc, tc, ext, pools, rep_tag=""):
    """Emit one full forward pass. `ext` holds external APs, `pools` the
    shared tile pools (so repeated bodies reuse SBUF/PSUM slots)."""
    sb, big, ps, pt_pool, small, dram = (
        pools["sb"], pools["big"], pools["ps"], pools["pt"], pools["small"],
        pools["dram"],
    )
    Exp = mybir.ActivationFunctionType.Exp
    mul = mybir.AluOpType.mult

    xs = ext["xs"]; wq_s = ext["wq_s"]; wk_s = ext["wk_s"]; wv_s = ext["wv_s"]
    wp_s = ext["wp_s"]; bqs = ext["bqs"]; bks = ext["bks"]; bvs = ext["bvs"]
    pbs = ext["pbs"]; tris = ext["tris"]; sels = ext["sels"]
    out_ext = ext["out"]

    # ---- working tiles for this pass ----
    qT = big.tile([128, 4, T], bf16, tag="qT")     # [d' slab, t]
    kT = big.tile([128, 4, T], bf16, tag="kT")
    v_all = big.tile([128, HL, 16, DH + 1], bf16, tag="v")  # [t-chunk part, h, tt, d+1]
    proj_dram = dram.tile([T, C], f32, tag="proj")
    sums_dram = dram.tile([32, 512], f32, tag="sums")
    if ext.get("norm_dma"):
        recips_dram_t = dram.tile([32, 512], bf16, tag="recips")
        ext["recips_dram"] = recips_dram_t
    rs_out = dram.tile([THALF, C], f32, tag="rs")

    nc.vector.memset(v_all[:, :, :, DH:DH + 1], 1.0)

    # ---- QKV projections ----
    # qT/kT: out[d' 128, t 512] = sum_cc w[:, cc, d'-slab].T @ xT[:, cc, t]
    for n in range(4):                  # t-chunks of 512
        for m in range(4):              # d' slabs of 128
            for w_s, dst, bias in ((wq_s, qT, bqs), (wk_s, kT, bks)):
                ps_t = ps.tile([128, 512], f32, tag="P")
                for cc in range(8):
                    nc.tensor.matmul(
                        ps_t[:],
                        lhsT=w_s[:, cc, m * 128:(m + 1) * 128],
                        rhs=xs[:, cc, n * 512:(n + 1) * 512],
                        start=(cc == 0), stop=(cc == 7),
                    )
                nc.vector.tensor_tensor(
                    dst[:, m, n * 512:(n + 1) * 512], ps_t[:],
                    bias[:, m:m + 1].to_broadcast([128, 512]),
                    mybir.AluOpType.add)
    # V: out[t 128, d' 512] = sum_cc xT[:, cc, tt].T @ wv[:, cc, :]
    for tt in range(16):
        ps_t = ps.tile([128, 512], f32, tag="P")
        for cc in range(8):
            nc.tensor.matmul(
                ps_t[:],
                lhsT=xs[:, cc, tt * 128:(tt + 1) * 128],
                rhs=wv_s[:, cc, :],
                start=(cc == 0), stop=(cc == 7),
            )
        nc.vector.tensor_tensor(
            v_all[:, :, tt, 0:DH],
            ps_t[:].rearrange("p (h d) -> p h d", h=HL),
            bvs[:].rearrange("p (h d) -> p h d", h=HL),
            mybir.AluOpType.add,
        )

    if "attn" in ext.get("skip", ()):
        # sink qkv outputs so DCE keeps the QKV phase
        snk = ext["snk"]
        nc.sync.dma_start(out=snk[0:128, 0:2048], in_=qT[:, 0, :])
        nc.sync.dma_start(out=snk[128:256, 0:2048], in_=kT[:, 0, :])
        nc.sync.dma_start(out=snk[256:384, 0:8320],
                          in_=v_all[:].rearrange("p a b c -> p (a b c)"))
        return

    # ---- attention ----
    for qt in range(4):
        attnT = sb.tile([128, 4, 512], bf16, tag="attnT")
        kmax = 4 * (qt + 1)
        for h in range(HL):
            j, half = h // 2, (h % 2) * 64
            ps_O = ps.tile([128, 512], f32, tag="O")
            for p in range(kmax // 2):
                kc0 = 2 * p
                # widths: diagonal chunks only need the causally-valid
                # query suffix (d = kc - 4*qt -> width 512 - 128*d)
                ws = []
                for e in range(2):
                    d = (kc0 + e) - 4 * qt
                    ws.append(512 if d < 0 else 512 - 128 * d)
                ps_S = ps.tile([128, 1024], f32, tag="S")
                for e in range(2):
                    kc, w = kc0 + e, ws[e]
                    nc.tensor.matmul(
                        ps_S[:, e * 512:e * 512 + w],
                        lhsT=kT[half:half + 64, j, kc * 128:(kc + 1) * 128],
                        rhs=qT[half:half + 64, j,
                               qt * 512 + (512 - w):(qt + 1) * 512],
                        start=True, stop=True,
                    )
                pt = pt_pool.tile([128, 1024], bf16, tag="pt")
                espan = 1024 if ext.get("wide_exp") else 512 + ws[1]
                nc.scalar.activation(pt[:, 0:espan], ps_S[:, 0:espan], Exp,
                                     scale=DH ** -0.5)
                if kc0 >= 4 * qt:        # diagonal pair -> causal mask
                    nc.vector.tensor_tensor(
                        pt[:, 0:espan], pt[:, 0:espan], tris[:, 0:espan], mul)
                for e in range(2):
                    kc, w = kc0 + e, ws[e]
                    nc.tensor.matmul(
                        ps_O[0:DH + 1, 512 - w:512],
                        lhsT=v_all[:, h, kc, :],
                        rhs=pt[:, e * 512:e * 512 + w],
                        start=(kc == 0), stop=(kc == kmax - 1),
                    )
            r = qt * 8 + h
            srow = small.tile([1, 512], f32, tag="srow")
            nc.vector.tensor_copy(srow[:], ps_O[DH:DH + 1, :])
            nc.sync.dma_start(out=sums_dram[r:r + 1, :], in_=srow[:])
            nc.vector.tensor_copy(attnT[half:half + 64, j, :], ps_O[0:64, :])

        if "proj" in ext.get("skip", ()):
            snk = ext["snk"]
            nc.sync.dma_start(out=snk[qt * 128:(qt + 1) * 128, 0:2048],
                              in_=attnT[:].rearrange("p a b -> p (a b)"))
            continue

        # normalization: recip of row-sums, partition-broadcast via a
        # selector matmul (rb[p, q] = recip[sel_row(p), q])
        ss = small.tile([8, 512], f32, tag="ss")
        nc.sync.dma_start(out=ss[:], in_=sums_dram[qt * 8:qt * 8 + 8, :])
        rr = small.tile([8, 512], bf16, tag="rr")
        with nc.allow_low_precision(reason="bf16 recip feeds bf16 matmul"):
            nc.vector.reciprocal(rr[:], ss[:])
        if ext.get("norm_dma"):
            recips_dram = ext["recips_dram"]
            nc.sync.dma_start(out=recips_dram[qt * 8:qt * 8 + 8, :], in_=rr[:])
            for j in range(4):
                rb = small.tile([128, 512], bf16, tag="rb")
                r0 = qt * 8 + 2 * j
                nc.sync.dma_start(
                    out=rb[0:64, :],
                    in_=recips_dram[r0:r0 + 1, :].to_broadcast([64, 512]))
                nc.sync.dma_start(
                    out=rb[64:128, :],
                    in_=recips_dram[r0 + 1:r0 + 2, :].to_broadcast([64, 512]))
                nc.vector.tensor_tensor(attnT[:, j, :], attnT[:, j, :], rb[:],
                                        mul)
        else:
            for j in range(4):
                rb_ps = ps.tile([128, 512], f32, tag="P")
                nc.tensor.matmul(rb_ps[:], lhsT=sels[:, j, :], rhs=rr[:],
                                 start=True, stop=True)
                nc.vector.tensor_tensor(attnT[:, j, :], attnT[:, j, :],
                                        rb_ps[:], mul)

        # output projection for this q-tile
        for tt in range(4):
            t0 = qt * 512 + tt * 128
            for cn in range(2):
                ps_P = ps.tile([128, 512], f32, tag="P")
                for j in range(4):
                    nc.tensor.matmul(
                        ps_P[:],
                        lhsT=attnT[:, j, tt * 128:(tt + 1) * 128],
                        rhs=wp_s[:, j, cn * 512:(cn + 1) * 512],
                        start=(j == 0), stop=(j == 3),
                    )
                ot = small.tile([128, 512], f32, tag="ot")
                nc.vector.tensor_tensor(
                    ot[:], ps_P[:], pbs[:, cn * 512:(cn + 1) * 512],
                    mybir.AluOpType.add)
                nc.sync.dma_start(
                    out=proj_dram[t0:t0 + 128, cn * 512:(cn + 1) * 512],
                    in_=ot[:])

        # pairwise ReduceScatter + output DMA for this q-tile's rows
        if ext.get("single_core") or ext.get("no_rs"):
            nc.sync.dma_start(
                out=out_ext[qt * 256:(qt + 1) * 256, :],
                in_=proj_dram[qt * 512:qt * 512 + 256, :])
        else:
            nc.gpsimd.collective_compute(
                "ReduceScatter",
                mybir.AluOpType.add,
                replica_groups=REPLICA_GROUPS,
                ins=[proj_dram[qt * 512:(qt + 1) * 512, :].opt()],
                outs=[rs_out[qt * 256:(qt + 1) * 256, :].opt()],
            )
            nc.sync.dma_start(
                out=out_ext[qt * 256:(qt + 1) * 256, :],
                in_=rs_out[qt * 256:(qt + 1) * 256, :])


def build_graph(reps=1, single_core=False, no_rs=False, skip=(),
                norm_dma=False, wide_exp=False, loop_n=0,
                pt_bufs=3, sb_bufs=2, small_bufs=2):
    nc = bacc.Bacc("TRN2", target_bir_lowering=False, debug=False,
                   num_devices=1 if single_core else 8)
    xT_e = nc.dram_tensor("xT", [C, T], bf16, kind="ExternalInput").ap()
    wq_e = nc.dram_tensor("wq", [C, CL], bf16, kind="ExternalInput").ap()
    wk_e = nc.dram_tensor("wk", [C, CL], bf16, kind="ExternalInput").ap()
    wv_e = nc.dram_tensor("wv", [C, CL], bf16, kind="ExternalInput").ap()
    wp_e = nc.dram_tensor("wp", [CL, C], bf16, kind="ExternalInput").ap()
    bq_e = nc.dram_tensor("bq", [128, 4], f32, kind="ExternalInput").ap()
    bk_e = nc.dram_tensor("bk", [128, 4], f32, kind="ExternalInput").ap()
    bv_e = nc.dram_tensor("bv", [1, CL], f32, kind="ExternalInput").ap()
    pb_e = nc.dram_tensor("pb", [1, C], f32, kind="ExternalInput").ap()
    tri_e = nc.dram_tensor("tri", [128, 1024], bf16, kind="ExternalInput").ap()
    sel_e = nc.dram_tensor("sel", [8, 4, 128], bf16, kind="ExternalInput").ap()
    out_e = nc.dram_tensor("out", [THALF, C], f32, kind="ExternalOutput").ap()
    snk_e = (nc.dram_tensor("snk", [512, 8320], bf16, kind="ExternalOutput").ap()
             if skip else None)

    with tile.TileContext(nc) as tc:
        with (
            tc.tile_pool(name="const", bufs=1) as const,
            tc.tile_pool(name="big", bufs=1) as big,
            tc.tile_pool(name="sb", bufs=sb_bufs) as sb,
            tc.tile_pool(name="pt", bufs=pt_bufs) as pt_pool,
            tc.tile_pool(name="small", bufs=small_bufs) as small,
            tc.tile_pool(name="ps", bufs=2, space="PSUM") as ps,
            tc.tile_pool(name="dram", bufs=2, space="DRAM") as dram,
        ):
            # load constants once
            xs = const.tile([128, 8, T], bf16, tag="xs")
            for cc in range(8):
                nc.sync.dma_start(
                    out=xs[:, cc, :],
                    in_=xT_e.rearrange("(c p) t -> p c t", p=128)[:, cc, :])
            wq_s = const.tile([128, 8, CL], bf16, tag="wq")
            wk_s = const.tile([128, 8, CL], bf16, tag="wk")
            wv_s = const.tile([128, 8, CL], bf16, tag="wv")
            for w_s, w_e in ((wq_s, wq_e), (wk_s, wk_e), (wv_s, wv_e)):
                for cc in range(8):
                    nc.sync.dma_start(
                        out=w_s[:, cc, :],
                        in_=w_e.rearrange("(c p) n -> p c n", p=128)[:, cc, :])
            wp_s = const.tile([128, 4, C], bf16, tag="wp")
            for cc in range(4):
                nc.sync.dma_start(
                    out=wp_s[:, cc, :],
                    in_=wp_e.rearrange("(c p) n -> p c n", p=128)[:, cc, :])
            bqs = const.tile([128, 4], f32, tag="bq")
            nc.sync.dma_start(out=bqs[:], in_=bq_e)
            bks = const.tile([128, 4], f32, tag="bk")
            nc.sync.dma_start(out=bks[:], in_=bk_e)
            bvs = const.tile([128, CL], f32, tag="bv")
            nc.sync.dma_start(out=bvs[:], in_=bv_e.to_broadcast([128, CL]))
            pbs = const.tile([128, C], f32, tag="pb")
            nc.sync.dma_start(out=pbs[:], in_=pb_e.to_broadcast([128, C]))
            tris = const.tile([128, 1024], bf16, tag="tri")
            nc.sync.dma_start(out=tris[:], in_=tri_e)
            sels = const.tile([8, 4, 128], bf16, tag="sel")
            nc.sync.dma_start(out=sels[:], in_=sel_e)

            ext = dict(xs=xs, wq_s=wq_s, wk_s=wk_s, wv_s=wv_s, wp_s=wp_s,
                       bqs=bqs, bks=bks, bvs=bvs, pbs=pbs, tris=tris,
                       sels=sels, out=out_e, snk=snk_e,
                       single_core=single_core, no_rs=no_rs, skip=skip,
                       norm_dma=norm_dma, wide_exp=wide_exp)
            pools = dict(sb=sb, big=big, ps=ps, pt=pt_pool, small=small,
                         dram=dram)
            if loop_n:
                hints = (mybir.EngineType.PE, mybir.EngineType.DVE,
                         mybir.EngineType.Activation, mybir.EngineType.SP,
                         mybir.EngineType.Pool)
                with tc.For_i(0, loop_n, 1, hint_engines=hints):
                    build_body(nc, tc, ext, pools)
            else:
                for r in range(reps):
                    build_body(nc, tc, ext, pools, rep_tag=str(r))

    nc.compile()
    return nc


def prep_shards(x, qkv_w, qkv_b, proj_w, proj_b):
    """Host-side sharding + layout prep. Returns in_maps for 8 cores."""
    kr = np.arange(128)[:, None]
    qr = np.arange(512)[None, :]
    tri1 = (qr >= kr)
    tri = np.concatenate([tri1, tri1], axis=1).astype(BF)
    sel = np.zeros((8, 4, 128), np.float32)
    for j in range(4):
        sel[2 * j, j, 0:64] = 1.0
        sel[2 * j + 1, j, 64:128] = 1.0
    sel = sel.astype(BF)

    x = np.asarray(x, np.float32)
    qkv_w = np.asarray(qkv_w, np.float32)
    qkv_b = np.asarray(qkv_b, np.float32)
    proj_w = np.asarray(proj_w, np.float32)
    proj_b = np.asarray(proj_b, np.float32)

    in_maps = []
    for core in range(8):
        b, g = core // 2, core % 2
        hsl = slice(g * CL, (g + 1) * CL)
        wq = qkv_w[0 * C:1 * C][hsl]
        wk = qkv_w[1 * C:2 * C][hsl]
        wv = qkv_w[2 * C:3 * C][hsl]
        in_maps.append({
            "xT": np.ascontiguousarray(x[b].T).astype(BF),
            "wq": np.ascontiguousarray(wq.T).astype(BF),
            "wk": np.ascontiguousarray(wk.T).astype(BF),
            "wv": np.ascontiguousarray(wv.T).astype(BF),
            "wp": np.ascontiguousarray(proj_w[:, hsl].T).astype(BF),
            "bq": np.ascontiguousarray(
                qkv_b[0 * C:1 * C][hsl].reshape(4, 128).T).astype(np.float32),
            "bk": np.ascontiguousarray(
                qkv_b[1 * C:2 * C][hsl].reshape(4, 128).T).astype(np.float32),
            "bv": qkv_b[2 * C:3 * C][hsl].reshape(1, CL).astype(np.float32),
            "pb": (proj_b if g == 0 else np.zeros_like(proj_b)
                   ).reshape(1, C).astype(np.float32),
            "tri": tri,
            "sel": sel,
        })
    return in_maps


def assemble(results):
    # chunked ReduceScatter: per q-tile chunk of 512 rows, rank 0 holds the
    # first 256 reduced rows, rank 1 the last 256
    out = np.empty((B, T, C), np.float32)
    for b in range(B):
        lo = results[2 * b]["out"]
        hi = results[2 * b + 1]["out"]
        for qt in range(4):
            out[b, qt * 512:qt * 512 + 256] = lo[qt * 256:(qt + 1) * 256]
            out[b, qt * 512 + 256:(qt + 1) * 512] = hi[qt * 256:(qt + 1) * 256]
    return out


_CACHE = {}


def _numpy_fallback(x, qkv_w, qkv_b, proj_w, proj_b, mask):
    x = np.asarray(x, np.float32)
    qkv = x @ np.asarray(qkv_w, np.float32).T + np.asarray(qkv_b, np.float32)
    qkv = qkv.reshape(B, T, 3, H, DH).transpose(2, 0, 3, 1, 4)
    q, k, v = qkv[0], qkv[1], qkv[2]
    att = np.einsum("bhqd,bhkd->bhqk", q, k) * (DH ** -0.5)
    att = np.where(np.asarray(mask), att, -np.inf)
    att = att - att.max(axis=-1, keepdims=True)
    att = np.exp(att)
    att /= att.sum(axis=-1, keepdims=True)
    o = np.einsum("bhqk,bhkd->bhqd", att, v)
    o = o.transpose(0, 2, 1, 3).reshape(B, T, C)
    return (o @ np.asarray(proj_w, np.float32).T
            + np.asarray(proj_b, np.float32)).astype(np.float32)


def kernel(x, qkv_w, qkv_b, proj_w, proj_b, mask):
    causal = np.tril(np.ones((T, T), dtype=bool))
    if not np.array_equal(np.asarray(mask).reshape(T, T), causal):
        return _numpy_fallback(x, qkv_w, qkv_b, proj_w, proj_b, mask)

    if "nc" not in _CACHE:
        _CACHE["nc"] = build_graph(reps=1)
    nc = _CACHE["nc"]
    in_maps = prep_shards(x, qkv_w, qkv_b, proj_w, proj_b)
    res = run_bass_kernel_spmd(nc, in_maps, core_ids=list(range(8)))
    return assemble(res.results)
